# revision 21
# baseline (speedup 1.0000x reference)
"""BiMamba3Block Trainium2 kernel.

Strategy: 8-way tensor-parallel over d_inner (SSM) / HID (MLP). Both batches
on every core. Two device launches; the cross-core partial-sum reductions,
layernorms and adaLN modulation (tiny) run on host between launches, which
avoids on-device collectives entirely.

Launch 1: in_proj -> causal depthwise conv -> silu -> x_proj -> dt/softplus
          -> per-n selective scan (DVE tensor_tensor_scan; bwd branch runs on
          reversed access patterns) -> y*silu(z) -> out_proj partial sums.
Launch 2: SwiGLU MLP partial sums over a HID shard.
"""
import os
import numpy as np
from contextlib import ExitStack

os.environ.setdefault("MYCRO_LOCAL_CACHE", "1")

NCORES = 8

# offload tuning: number of n-values whose tmp-mul/Y-add run on GPSIMD,
# and whether conv taps run on GPSIMD
GP_N = 6
GP_CONV = False

FULL = dict(B=2, L=1024, D=1024, COND=1024, dI=2048, N=16, R=64, K=4, HID=2048)


def _dims(cfg, ncores):
    d = dict(cfg)
    d["CH"] = d["dI"] // ncores          # d_inner slice per core per branch
    d["HCH"] = d["HID"] // ncores        # HID slice per core
    d["TOK"] = d["B"] * d["L"]
    d["TS"] = min(512, d["TOK"])         # tok slice for moving operands
    return d


# ---------------------------------------------------------------------------
# device kernel builders
# ---------------------------------------------------------------------------

def _rev_whole(t):
    """Reversed view of the full free dim of tile t."""
    a = t[:].copy()
    st, cnt = a.ap[-1]
    a.offset = a.offset + (cnt - 1) * st
    a.ap = a.ap[:-1] + [[-st, cnt]]
    return a


def _rev_batch(t, b, L):
    """Reversed-time view of t[:, b*L:(b+1)*L] (negative free step)."""
    a = t[:, b * L:(b + 1) * L].copy()
    st, cnt = a.ap[-1]
    a.offset = a.offset + (cnt - 1) * st
    a.ap = a.ap[:-1] + [[-st, cnt]]
    return a


def build_l1(dims, A_rows):
    """A_rows: dict br -> list of N floats (negative)."""
    import concourse.tile as tile
    from concourse import bacc, mybir
    from concourse._compat import with_exitstack

    F16, F32 = mybir.dt.float16, mybir.dt.float32
    ALU = mybir.AluOpType
    AF = mybir.ActivationFunctionType

    B, L, D, CH, N, R, K = (dims[k] for k in ("B", "L", "D", "CH", "N", "R", "K"))
    TOK, TS = dims["TOK"], dims["TS"]
    NCT = CH // 128            # channel tiles per branch
    NKT = D // 128             # contraction tiles of D
    NTS = TOK // TS            # tok slices
    NTC = TOK // 128           # tok chunks (out_proj stationary)
    NHALF = max(1, D // 512)   # out_proj free-dim splits
    HW_ = D // NHALF
    BRS = ("f", "b")

    nc = bacc.Bacc("TRN2", target_bir_lowering=False, debug=False)

    h1T_d = nc.dram_tensor("h1T", [D, TOK], F16, kind="ExternalInput").ap()
    din = {}
    for br in BRS:
        din[br] = {
            "inW": nc.dram_tensor(f"inW_{br}", [D, 2 * CH], F16, kind="ExternalInput").ap(),
            "convw": nc.dram_tensor(f"convw_{br}", [128, NCT * K], F32, kind="ExternalInput").ap(),
            "convb": nc.dram_tensor(f"convb_{br}", [128, NCT], F32, kind="ExternalInput").ap(),
            "xprojW": nc.dram_tensor(f"xprojW_{br}", [CH, R + 2 * N], F16, kind="ExternalInput").ap(),
            "dtW": nc.dram_tensor(f"dtW_{br}", [R, CH], F16, kind="ExternalInput").ap(),
            "dtb": nc.dram_tensor(f"dtb_{br}", [128, NCT], F32, kind="ExternalInput").ap(),
            "Dvec": nc.dram_tensor(f"Dvec_{br}", [128, NCT], F32, kind="ExternalInput").ap(),
            "outW": nc.dram_tensor(f"outW_{br}", [CH, D], F16, kind="ExternalInput").ap(),
        }
    part1_d = nc.dram_tensor("part1", [TOK, D], F32, kind="ExternalOutput").ap()

    def kern(ctx: ExitStack, tc: tile.TileContext):
        nc = tc.nc
        wpool = ctx.enter_context(tc.tile_pool(name="weights", bufs=1))
        big = ctx.enter_context(tc.tile_pool(name="big", bufs=1))
        work = ctx.enter_context(tc.tile_pool(name="work", bufs=2))
        scanp = ctx.enter_context(tc.tile_pool(name="scanp", bufs=2))
        dramp = ctx.enter_context(tc.tile_pool(name="dram", bufs=1, space="DRAM"))
        ps_in = ctx.enter_context(tc.tile_pool(name="ps_in", bufs=2, space="PSUM"))
        ps_sm = ctx.enter_context(tc.tile_pool(name="ps_sm", bufs=2, space="PSUM"))
        ps_out = ctx.enter_context(tc.tile_pool(name="ps_out", bufs=2, space="PSUM"))

        # ---- load h1T + weights -------------------------------------------
        # "h1t" tag (4KB slots, 8 bufs): h1T tiles, later reused for
        # u_cm/w_cm/dbcT/bc16. "inW" tag (1KB slots, 16 bufs).
        h1T_s = []
        for kt in range(NKT):
            t = big.tile([128, TOK], F16, tag="h1t", bufs=NKT, name=f"h1t{kt}")
            nc.sync.dma_start(t[:], h1T_d[kt * 128:(kt + 1) * 128, :])
            h1T_s.append(t)
        inW_s = {}
        for br in BRS:
            for kt in range(NKT):
                t = big.tile([128, 2 * CH], F16, tag="inW", bufs=2 * NKT,
                             name=f"inW_{br}{kt}")
                nc.sync.dma_start(t[:], din[br]["inW"][kt * 128:(kt + 1) * 128, :])
                inW_s[(br, kt)] = t

        smalls = {}
        for br in BRS:
            for nm, wd in (("convw", NCT * K), ("convb", NCT), ("dtb", NCT), ("Dvec", NCT)):
                t = wpool.tile([128, wd], F32, tag=f"sm_{nm}_{br}", name=f"sm_{nm}_{br}")
                nc.sync.dma_start(t[:], din[br][nm][:])
                smalls[(br, nm)] = t
            for j in range(NCT):
                t = wpool.tile([128, R + 2 * N], F16, tag=f"xprojW_{br}{j}",
                               name=f"xprojW_{br}{j}")
                nc.sync.dma_start(t[:], din[br]["xprojW"][j * 128:(j + 1) * 128, :])
                smalls[(br, "xprojW", j)] = t
            t = wpool.tile([R, CH], F16, tag=f"dtW_{br}", name=f"dtW_{br}")
            nc.sync.dma_start(t[:], din[br]["dtW"][:])
            smalls[(br, "dtW")] = t
            for j in range(NCT):
                t = wpool.tile([128, D], F16, tag=f"outW_{br}{j}", name=f"outW_{br}{j}")
                nc.sync.dma_start(t[:], din[br]["outW"][j * 128:(j + 1) * 128, :])
                smalls[(br, "outW", j)] = t

        # ---- in_proj x_in + conv (per branch), then z, then per-branch scan
        PAD = K - 1
        xpad = {}
        sz = {}
        u_cm = {}

        def inproj_cols(br, cc_list):
            for cc in cc_list:
                for ts in range(NTS):
                    ps = ps_in.tile([128, TS], F32, tag="psin")
                    for kt in range(NKT):
                        nc.tensor.matmul(
                            ps[:], inW_s[(br, kt)][:, cc * 128:(cc + 1) * 128],
                            h1T_s[kt][:, ts * TS:(ts + 1) * TS],
                            start=(kt == 0), stop=(kt == NKT - 1))
                    if cc < NCT:
                        b0 = (ts * TS) // L
                        off = PAD + (ts * TS) % L
                        nc.scalar.copy(xpad[(br, cc, b0)][:, off:off + TS], ps[:])
                    else:
                        j = cc - NCT
                        if NATIVE_SILU:
                            nc.scalar.activation(
                                sz[(br, j)][:, ts * TS:(ts + 1) * TS], ps[:], AF.Silu)
                        else:
                            sg = work.tile([128, TS], F16, tag="sgz")
                            nc.scalar.activation(sg[:], ps[:], AF.Sigmoid)
                            nc.vector.tensor_tensor(
                                sz[(br, j)][:, ts * TS:(ts + 1) * TS], ps[:], sg[:],
                                ALU.mult)

        def conv_u(br):
            cw = smalls[(br, "convw")]
            cb = smalls[(br, "convb")]
            for j in range(NCT):
                u_cm[(br, j)] = big.tile([128, TOK], F16, tag=f"u{j}", bufs=2,
                                         name=f"u_{br}{j}")
                for b in range(B):
                    xp = xpad[(br, j, b)]
                    # fwd: u[t]=sum_tap k[tap]*x[t-3+tap]; bwd: k[tap]*x[t+3-tap]
                    tap0 = 0 if br == "f" else 2 * PAD
                    acc = work.tile([128, L], F16, tag="conv_acc")
                    nc.vector.tensor_scalar(
                        acc[:], xp[:, tap0:tap0 + L],
                        cw[:, j * K:j * K + 1], cb[:, j:j + 1], ALU.mult, ALU.add)
                    for tap in range(1, K):
                        o = tap if br == "f" else 2 * PAD - tap
                        acc2 = work.tile([128, L], F16, tag="conv_acc")
                        nc.vector.scalar_tensor_tensor(
                            acc2[:], xp[:, o:o + L], cw[:, j * K + tap:j * K + tap + 1],
                            acc[:], ALU.mult, ALU.add)
                        acc = acc2
                    if NATIVE_SILU:
                        nc.scalar.activation(
                            u_cm[(br, j)][:, b * L:(b + 1) * L], acc[:], AF.Silu)
                    else:
                        sg = work.tile([128, L], F16, tag="sgu")
                        nc.scalar.activation(sg[:], acc[:], AF.Sigmoid)
                        nc.vector.tensor_tensor(
                            u_cm[(br, j)][:, b * L:(b + 1) * L], acc[:], sg[:], ALU.mult)

        dbcT = {}
        bcD = {}

        def xproj(br):
            dbcT[br] = big.tile([R + 2 * N, TOK], F16, tag="dbcT", bufs=1,
                                name=f"dbcT_{br}")
            for ts in range(NTS):
                ps = ps_sm.tile([128, TS], F32, tag="ps_small")
                for j in range(NCT):
                    nc.tensor.matmul(
                        ps[0:R + 2 * N, :], smalls[(br, "xprojW", j)][:],
                        u_cm[(br, j)][:, ts * TS:(ts + 1) * TS],
                        start=(j == 0), stop=(j == NCT - 1))
                nc.scalar.copy(dbcT[br][:, ts * TS:(ts + 1) * TS], ps[0:R + 2 * N, :])
            bcD[br] = dramp.tile([2 * N, TOK], F16, tag=f"bcD_{br}", name=f"bcD_{br}")
            nc.sync.dma_start(bcD[br][:], dbcT[br][R:R + 2 * N, :])

        def bcast_row(br, dst, row):
            src = bcD[br][row:row + 1, :].copy()
            src.ap = [[0, 128]] + src.ap[1:]
            nc.sync.dma_start(dst, src)

        def dt_w(br):
            dt_cm = [big.tile([128, TOK], F16, tag=f"dt_cm{j}", name=f"dt_{br}{j}")
                     for j in range(NCT)]
            w_cm = [big.tile([128, TOK], F16, tag=f"w{j}", bufs=1, name=f"w_{br}{j}")
                    for j in range(NCT)]
            dw = smalls[(br, "dtW")]
            dtb = smalls[(br, "dtb")]
            for j in range(NCT):
                for ts in range(NTS):
                    ps = ps_sm.tile([128, TS], F32, tag="ps_small")
                    nc.tensor.matmul(
                        ps[:], dw[:, j * 128:(j + 1) * 128],
                        dbcT[br][0:R, ts * TS:(ts + 1) * TS],
                        start=True, stop=True)
                    ex = work.tile([128, TS], F32, tag="dt_exp", bufs=1)
                    nc.scalar.activation(ex[:], ps[:], AF.Exp, bias=dtb[:, j:j + 1])
                    nc.scalar.activation(
                        dt_cm[j][:, ts * TS:(ts + 1) * TS], ex[:], AF.Ln, bias=1.0)
            for j in range(NCT):
                nc.vector.tensor_tensor(w_cm[j][:], dt_cm[j][:], u_cm[(br, j)][:],
                                        ALU.mult)
            return dt_cm, w_cm

        y_cm = {}

        def scans_y(br, dt_cm, w_cm):
            # one scan op per n covers both batch segments; the first-processed
            # column of the second segment only multiplies the zero initial
            # state, so zeroing it in dA makes the concatenation exact.
            A_br = A_rows[br]
            Yacc = {j: None for j in range(NCT)}
            for n in range(N):
                Bb = scanp.tile([128, TOK], F16, tag="Bb", bufs=2)
                bcast_row(br, Bb[:], n)
                Cb = scanp.tile([128, TOK], F16, tag="Cb", bufs=2)
                bcast_row(br, Cb[:], N + n)
                for j in range(NCT):
                    dA = scanp.tile([128, TOK], F16, tag="dA", bufs=2)
                    nc.scalar.activation(dA[:], dt_cm[j][:], AF.Exp, scale=float(A_br[n]))
                    for b in range(1, B):
                        col = b * L if br == "f" else b * L - 1
                        nc.scalar.activation(dA[:, col:col + 1], dA[:, col:col + 1],
                                             AF.Copy, scale=0.0)
                    bB = scanp.tile([128, TOK], F16, tag="bB", bufs=2)
                    nc.vector.tensor_tensor(bB[:], w_cm[j][:], Bb[:], ALU.mult)
                    h = scanp.tile([128, TOK], F16, tag="h", bufs=2)
                    if br == "f":
                        nc.vector.tensor_tensor_scan(
                            h[:], dA[:], bB[:], 0.0, ALU.mult, ALU.add)
                    else:
                        nc.vector.tensor_tensor_scan(
                            _rev_whole(h), _rev_whole(dA), _rev_whole(bB),
                            0.0, ALU.mult, ALU.add)
                    if n == 0:
                        Ynew = scanp.tile([128, TOK], F16, tag=f"Y{j}", name=f"Y{j}")
                        nc.vector.tensor_tensor(Ynew[:], h[:], Cb[:], ALU.mult)
                    else:
                        tmp = scanp.tile([128, TOK], F16, tag="Cb", bufs=2, name="tmp")
                        nc.vector.tensor_tensor(tmp[:], h[:], Cb[:], ALU.mult)
                        Ynew = scanp.tile([128, TOK], F16, tag=f"Y{j}", name=f"Y{j}")
                        nc.vector.tensor_tensor(Ynew[:], Yacc[j][:], tmp[:], ALU.add)
                    Yacc[j] = Ynew
            Dv = smalls[(br, "Dvec")]
            for j in range(NCT):
                t1 = work.tile([128, TOK], F16, tag="t1")
                nc.vector.scalar_tensor_tensor(
                    t1[:], u_cm[(br, j)][:], Dv[:, j:j + 1], Yacc[j][:],
                    ALU.mult, ALU.add)
                yt = big.tile([128, TOK], F16, tag=f"y_cm_{br}{j}", name=f"y_cm_{br}{j}")
                nc.vector.tensor_tensor(yt[:], t1[:], sz[(br, j)][:], ALU.mult)
                y_cm[(br, j)] = yt

        # program order chosen so the forward branch's scan chain starts as
        # early as possible while branch-b prep fills PE/ACT idle time.
        for br in BRS:
            for j in range(NCT):
                sz[(br, j)] = big.tile([128, TOK], F16, tag=f"sz_{br}{j}",
                                       name=f"sz_{br}{j}")

        def alloc_xpads(br):
            for j in range(NCT):
                for b in range(B):
                    t = big.tile([128, L + 2 * PAD], F16, tag=f"xpad{j}_{b}",
                                 bufs=1, name=f"xpad_{br}{j}_{b}")
                    nc.vector.memset(t[:, :PAD], 0.0)
                    nc.vector.memset(t[:, PAD + L:], 0.0)
                    xpad[(br, j, b)] = t

        alloc_xpads("f")
        inproj_cols("f", range(NCT))
        conv_u("f")
        alloc_xpads("b")
        inproj_cols("b", range(NCT))
        conv_u("b")
        xproj("f")
        dtf, wf = dt_w("f")
        inproj_cols("f", range(NCT, 2 * NCT))
        inproj_cols("b", range(NCT, 2 * NCT))
        scans_y("f", dtf, wf)
        xproj("b")
        dtb_, wb = dt_w("b")
        scans_y("b", dtb_, wb)

        # ---- out_proj: part1[tc] = sum_{br,j} y^T @ outW ------------------
        for tcn in range(NTC):
            ps = ps_out.tile([128, D], F32, tag="psout")
            mms = [(br, j, half) for br in BRS for j in range(NCT) for half in range(NHALF)]
            nmm = len(mms)
            for i, (br, j, half) in enumerate(mms):
                nc.tensor.matmul(
                    ps[:, half * HW_:(half + 1) * HW_],
                    y_cm[(br, j)][:, tcn * 128:(tcn + 1) * 128],
                    smalls[(br, "outW", j)][:, half * HW_:(half + 1) * HW_],
                    start=(i < NHALF), stop=(i >= nmm - NHALF))
            ost = work.tile([128, D], F32, tag="t1")
            nc.scalar.copy(ost[:], ps[:])
            nc.sync.dma_start(part1_d[tcn * 128:(tcn + 1) * 128, :], ost[:])

    wk = with_exitstack(kern)
    with tile.TileContext(nc) as tc:
        wk(tc)
    nc.compile()
    return nc


def build_l2(dims):
    import concourse.tile as tile
    from concourse import bacc, mybir
    from concourse._compat import with_exitstack

    F16, F32 = mybir.dt.float16, mybir.dt.float32
    ALU = mybir.AluOpType
    AF = mybir.ActivationFunctionType

    D, HCH = dims["D"], dims["HCH"]
    TOK, TS = dims["TOK"], dims["TS"]
    NKT = D // 128
    NHT = HCH // 128
    NTS = TOK // TS
    NTC = TOK // 128
    NHALF = max(1, D // 512)
    HW_ = D // NHALF

    nc = bacc.Bacc("TRN2", target_bir_lowering=False, debug=False)
    h2T_d = nc.dram_tensor("h2T", [D, TOK], F16, kind="ExternalInput").ap()
    w1_d = nc.dram_tensor("w1s", [D, HCH], F16, kind="ExternalInput").ap()
    w2_d = nc.dram_tensor("w2s", [D, HCH], F16, kind="ExternalInput").ap()
    w3_d = nc.dram_tensor("w3s", [HCH, D], F16, kind="ExternalInput").ap()
    part2_d = nc.dram_tensor("part2", [TOK, D], F32, kind="ExternalOutput").ap()

    def kern(ctx: ExitStack, tc: tile.TileContext):
        nc = tc.nc
        wpool = ctx.enter_context(tc.tile_pool(name="weights", bufs=1))
        big = ctx.enter_context(tc.tile_pool(name="big", bufs=1))
        work = ctx.enter_context(tc.tile_pool(name="work", bufs=4))
        ps_a = ctx.enter_context(tc.tile_pool(name="ps_a", bufs=2, space="PSUM"))
        ps_o = ctx.enter_context(tc.tile_pool(name="ps_o", bufs=2, space="PSUM"))

        h2T_s, w1_s, w2_s, w3_s = [], [], [], []
        for kt in range(NKT):
            t = wpool.tile([128, TOK], F16, tag=f"h2t{kt}")
            nc.sync.dma_start(t[:], h2T_d[kt * 128:(kt + 1) * 128, :])
            h2T_s.append(t)
            t = wpool.tile([128, HCH], F16, tag=f"w1_{kt}")
            nc.sync.dma_start(t[:], w1_d[kt * 128:(kt + 1) * 128, :])
            w1_s.append(t)
            t = wpool.tile([128, HCH], F16, tag=f"w2_{kt}")
            nc.sync.dma_start(t[:], w2_d[kt * 128:(kt + 1) * 128, :])
            w2_s.append(t)
        for j in range(NHT):
            t = wpool.tile([128, D], F16, tag=f"w3_{j}")
            nc.sync.dma_start(t[:], w3_d[j * 128:(j + 1) * 128, :])
            w3_s.append(t)

        g_cm = [big.tile([128, TOK], F16, tag=f"g_cm{j}") for j in range(NHT)]
        for j in range(NHT):
            for ts in range(NTS):
                psa = ps_a.tile([128, TS], F32, tag="psa")
                psb = ps_a.tile([128, TS], F32, tag="psb")
                for kt in range(NKT):
                    nc.tensor.matmul(psa[:], w1_s[kt][:, j * 128:(j + 1) * 128],
                                     h2T_s[kt][:, ts * TS:(ts + 1) * TS],
                                     start=(kt == 0), stop=(kt == NKT - 1))
                for kt in range(NKT):
                    nc.tensor.matmul(psb[:], w2_s[kt][:, j * 128:(j + 1) * 128],
                                     h2T_s[kt][:, ts * TS:(ts + 1) * TS],
                                     start=(kt == 0), stop=(kt == NKT - 1))
                sg = work.tile([128, TS], F32, tag="sg")
                nc.scalar.activation(sg[:], psa[:], AF.Sigmoid)
                t1 = work.tile([128, TS], F16, tag="t1")
                nc.vector.tensor_tensor(t1[:], psa[:], sg[:], ALU.mult)
                nc.vector.tensor_tensor(
                    g_cm[j][:, ts * TS:(ts + 1) * TS], t1[:], psb[:], ALU.mult)

        for tcn in range(NTC):
            ps = ps_o.tile([128, D], F32, tag="pso")
            mms = [(j, half) for j in range(NHT) for half in range(NHALF)]
            nmm = len(mms)
            for i, (j, half) in enumerate(mms):
                nc.tensor.matmul(
                    ps[:, half * HW_:(half + 1) * HW_],
                    g_cm[j][:, tcn * 128:(tcn + 1) * 128],
                    w3_s[j][:, half * HW_:(half + 1) * HW_],
                    start=(i < NHALF), stop=(i >= nmm - NHALF))
            nc.sync.dma_start(part2_d[tcn * 128:(tcn + 1) * 128, :], ps[:])

    wk = with_exitstack(kern)
    with tile.TileContext(nc) as tc:
        wk(tc)
    nc.compile()
    return nc


# ---------------------------------------------------------------------------
# host side
# ---------------------------------------------------------------------------

EPS = 1e-5


def _adaln(x64, c64, W, b):
    mod = c64 @ np.asarray(W, np.float64) + np.asarray(b, np.float64)
    D3 = mod.shape[-1] // 3
    shift, scale, gate = mod[:, :D3], mod[:, D3:2 * D3], mod[:, 2 * D3:]
    mu = x64.mean(-1, keepdims=True)
    var = x64.var(-1, keepdims=True)
    normed = (x64 - mu) / np.sqrt(var + EPS)
    h = normed * (1.0 + scale[:, None, :]) + shift[:, None, :]
    return h, gate


def _prep_l1_inmaps(dims, h1T, fwd_params, bwd_params, ncores):
    CH, N, R, K = (dims[k] for k in ("CH", "N", "R", "K"))
    NCT = CH // 128
    in_maps = [dict() for _ in range(ncores)]
    for cid in range(ncores):
        in_maps[cid]["h1T"] = h1T
    for br, p in (("f", fwd_params), ("b", bwd_params)):
        in_W = np.asarray(p["in_W"], np.float32)
        dI = in_W.shape[1] // 2
        for cid in range(ncores):
            lo, hi = cid * CH, (cid + 1) * CH
            m = in_maps[cid]
            m[f"inW_{br}"] = np.ascontiguousarray(
                np.concatenate([in_W[:, lo:hi], in_W[:, dI + lo:dI + hi]],
                               axis=1)).astype(np.float16)
            cw = np.asarray(p["conv_w"], np.float32)[lo:hi].reshape(NCT, 128, K)
            m[f"convw_{br}"] = np.ascontiguousarray(
                cw.transpose(1, 0, 2).reshape(128, NCT * K))
            for nm, vec in (("convb", p["conv_b"]), ("dtb", p["dt_b"]), ("Dvec", p["D"])):
                m[f"{nm}_{br}"] = np.ascontiguousarray(
                    np.asarray(vec, np.float32)[lo:hi].reshape(NCT, 128).T)
            m[f"xprojW_{br}"] = np.asarray(p["xproj_W"], np.float32)[lo:hi].astype(np.float16)
            m[f"dtW_{br}"] = np.ascontiguousarray(
                np.asarray(p["dt_W"], np.float32)[:, lo:hi]).astype(np.float16)
            m[f"outW_{br}"] = np.asarray(p["out_W"], np.float32)[lo:hi].astype(np.float16)
    return in_maps


_CACHE = {}

PROFILE = False
LAST_EXEC_NS = []
LAST_RESULTS = []


def _hw_runner(nc, in_maps, ncores):
    from concourse.bass_utils import run_bass_kernel_spmd
    res = run_bass_kernel_spmd(nc, in_maps, core_ids=list(range(ncores)),
                               trace=PROFILE)
    LAST_RESULTS.append(res)
    if res.exec_time_ns is not None:
        LAST_EXEC_NS.append(res.exec_time_ns)
    return res.results


def kernel(x, c, fwd_params, bwd_params, adaln1_W, adaln1_b,
           adaln2_W, adaln2_b, mlp_w1, mlp_w2, mlp_w3):
    return _forward(x, c, fwd_params, bwd_params, adaln1_W, adaln1_b,
                    adaln2_W, adaln2_b, mlp_w1, mlp_w2, mlp_w3,
                    cfg=FULL, ncores=NCORES, runner=_hw_runner)


def _forward(x, c, fwd_params, bwd_params, adaln1_W, adaln1_b,
             adaln2_W, adaln2_b, mlp_w1, mlp_w2, mlp_w3,
             cfg, ncores, runner):
    dims = _dims(cfg, ncores)
    B, L, D, TOK = dims["B"], dims["L"], dims["D"], dims["TOK"]

    x = np.asarray(x, np.float32)
    c64 = np.asarray(c, np.float64)
    x64 = x.astype(np.float64)

    A_rows = {}
    for br, p in (("f", fwd_params), ("b", bwd_params)):
        Alog = np.asarray(p["A_log"], np.float64)
        A_rows[br] = [float(v) for v in -np.exp(Alog[0])]

    key = tuple(sorted(dims.items()))
    if key not in _CACHE:
        _CACHE[key] = (build_l1(dims, A_rows), build_l2(dims))
    nc1, nc2 = _CACHE[key]

    # ---- phase 1 host: adaln1 ----
    h1, gate1 = _adaln(x64, c64, adaln1_W, adaln1_b)
    h1T = np.ascontiguousarray(h1.reshape(TOK, D).T, dtype=np.float16)

    in_maps = _prep_l1_inmaps(dims, h1T, fwd_params, bwd_params, ncores)
    res1 = runner(nc1, in_maps, ncores)
    merged = np.zeros((TOK, D), np.float64)
    for r in res1:
        merged += r["part1"].astype(np.float64)
    merged = merged.reshape(B, L, D)

    # ---- phase 2 host: residual + adaln2 ----
    x2 = x64 + gate1[:, None, :] * merged
    h2, gate2 = _adaln(x2, c64, adaln2_W, adaln2_b)
    h2T = np.ascontiguousarray(h2.reshape(TOK, D).T, dtype=np.float16)

    HCH = dims["HCH"]
    in_maps2 = []
    for cid in range(ncores):
        lo, hi = cid * HCH, (cid + 1) * HCH
        in_maps2.append({
            "h2T": h2T,
            "w1s": np.ascontiguousarray(np.asarray(mlp_w1, np.float32)[:, lo:hi]).astype(np.float16),
            "w2s": np.ascontiguousarray(np.asarray(mlp_w2, np.float32)[:, lo:hi]).astype(np.float16),
            "w3s": np.asarray(mlp_w3, np.float32)[lo:hi].astype(np.float16),
        })
    res2 = runner(nc2, in_maps2, ncores)
    mlp_out = np.zeros((TOK, D), np.float64)
    for r in res2:
        mlp_out += r["part2"].astype(np.float64)
    mlp_out = mlp_out.reshape(B, L, D)

    out = x2 + gate2[:, None, :] * mlp_out
    return out.astype(np.float32)
